# revision 1
# baseline (speedup 1.0000x reference)
"""Dynamic per-pixel 3x3 filtering on 8 Trainium2 NeuronCores.

out[b,c,y,x] = sum_{ki,kj} img[b,c,y+ki-1,x+kj-1] * kernels[b,c,ki*3+kj,y,x]
(zero padding outside the image).

Sharding: pure data parallel, one batch sample per core (B=8, 8 cores).

Per-core layout: each channel's [512, 512] image plane is viewed as
[128 partitions, 4 blocks, 512 cols] (row r = block*128 + partition).
Row-shifted variants (y-1 / y+1) are built ON-CHIP via the idle
TensorE: the plane is transposed in 128x128 chunks into a zero-padded
"rows in the free dim" tile, then transposed back with a +-1 free-dim
offset (a partition shift is impossible on the lockstep compute engines,
SBUF->SBUF partition-shift DMAs serialize onto one SDMA engine, and
re-reading shifted rows from HBM costs 2.5 MB/channel of the bottleneck
HBM bandwidth). ScalarE evacuates PSUM. Column shifts are free-dim AP
offsets.

All 17 elementwise passes (9 mult + 8 accumulate) run on the Vector
engine: concurrent GPSIMD tensor_tensor work contends with DVE for the
shared SBUF port (measured 2.5x DVE slowdown), so a tap split across
engines loses. DMA issue is split across both HWDGE sequencers (SP for
image/shift traffic, ACT for kernel-tile loads and stores) because a
single sequencer serializes on per-DMA descriptor generation.
"""

from contextlib import ExitStack

import numpy as np

import concourse.bacc as bacc
import concourse.mybir as mybir
import concourse.tile as tile
from concourse import masks
from concourse.bass_utils import run_bass_kernel_spmd

C, H, W = 3, 512, 512
KK = 9
NCORES = 8
P = 128
NB = H // P          # 4 row blocks per channel
FW = NB * W          # 2048 free-dim width of a channel mega-tile
F32 = mybir.dt.float32

# Taps: t = ki*3 + kj; row shift = ki-1 (top/mid/bot), col shift = kj-1.
# mid taps first (no shift-DMA dependency); first tap must be dx=0 (full write).
TAP_ORDER = [4, 3, 5, 1, 0, 2, 7, 6, 8]
# For all channels but the last, the taps run as TWO independent accumulator
# chains (one less DVE pass than a single 17-pass chain); the chains are
# merged by a CCE-accumulating store that fires mid-kernel. The last channel
# stays single-chain so the kernel tail is a plain (fast) store.
CHAIN_A = [4, 3, 5, 1]
CHAIN_B = [7, 0, 2, 6, 8]


def _r3(ap):
    """[128, FW] -> [128, NB, W] block view of a channel mega-tile."""
    return ap.rearrange("p (b x) -> p b x", x=W)


def _emit(nc, tc, ctx):
    img = nc.dram_tensor("img", (C, H, W), F32, kind="ExternalInput").ap()
    ker = nc.dram_tensor("kernels", (C, KK, H, W), F32, kind="ExternalInput").ap()
    out = nc.dram_tensor("out", (C, H, W), F32, kind="ExternalOutput").ap()

    v_pool = ctx.enter_context(tc.tile_pool(name="v", bufs=2))
    k_pool = ctx.enter_context(tc.tile_pool(name="k", bufs=11))
    acc_pool = ctx.enter_context(tc.tile_pool(name="acc", bufs=2))
    tmp_pool = ctx.enter_context(tc.tile_pool(name="tmp", bufs=1))
    t_pool = ctx.enter_context(tc.tile_pool(name="tp", bufs=2))
    ps_pool = ctx.enter_context(tc.tile_pool(name="ps", bufs=8, space="PSUM"))
    id_pool = ctx.enter_context(tc.tile_pool(name="ident", bufs=1))

    ident = id_pool.tile([P, P], F32, tag="ident")
    masks.make_identity(nc, ident[:, :])
    store_sem = nc.alloc_semaphore("store_order")
    NXC = W // P          # 4 col-chunks of 128
    TS = H + 2            # 514: transposed row axis incl. zero pads

    for c in range(C):
        # img rows for this channel: mid[p, b*W + x] = img[c, b*128 + p, x].
        # Loaded per block so the transpose chain (which feeds top/bot)
        # starts after the first 0.25 MB instead of the full 1 MB.
        rows = img[c].rearrange("(b p) x -> p b x", p=P)
        mid = v_pool.tile([P, FW], F32, tag="mid")
        for b in range(NB):
            nc.sync.dma_start(mid[:, b * W : (b + 1) * W], rows[:, b, :])
        kts = {}
        for t in TAP_ORDER:
            kt = k_pool.tile([P, FW], F32, tag="kt")
            nc.scalar.dma_start(
                _r3(kt[:, :]), ker[c, t].rearrange("(b p) x -> p b x", p=P)
            )
            kts[t] = kt

        # Transposed plane: T[xp, xc*TS + 1 + r] = img[c, r, xc*128 + xp],
        # with zero columns at slot 0 (row -1) and slot 513 (row 512).
        T = t_pool.tile([P, NXC * TS], F32, tag="T")
        for xc in range(NXC):
            nc.scalar.memzero(T[:, xc * TS : xc * TS + 1])
            nc.scalar.memzero(T[:, xc * TS + TS - 1 : xc * TS + TS])
        for b in range(NB):
            for xc in range(NXC):
                ps = ps_pool.tile([P, P], F32, tag="ps")
                nc.tensor.transpose(
                    ps[:, :], mid[:, b * W + xc * P : b * W + (xc + 1) * P],
                    ident[:, :],
                )
                nc.scalar.copy(
                    T[:, xc * TS + 1 + b * P : xc * TS + 1 + (b + 1) * P],
                    ps[:, :],
                )
        # top[q, b*512 + xc*128 + xp] = img row (128b + q - 1) -> transpose
        # back from T with free offset 0; bot with free offset 2.
        top = v_pool.tile([P, FW], F32, tag="top")
        bot = v_pool.tile([P, FW], F32, tag="bot")
        for dst, off in ((top, 0), (bot, 2)):
            for b in range(NB):
                for xc in range(NXC):
                    ps = ps_pool.tile([P, P], F32, tag="ps")
                    nc.tensor.transpose(
                        ps[:, :],
                        T[:, xc * TS + off + b * P : xc * TS + off + (b + 1) * P],
                        ident[:, :],
                    )
                    nc.scalar.copy(
                        dst[:, b * W + xc * P : b * W + (xc + 1) * P], ps[:, :]
                    )


        tmp = tmp_pool.tile([P, FW], F32, tag="tmp")
        vs = [top, mid, bot]
        eng = nc.vector

        def run_chain(taps, tag):
            a = acc_pool.tile([P, FW], F32, tag=tag)
            first = True
            for t in taps:
                ki, kj = divmod(t, 3)
                v, dx = vs[ki], kj - 1
                if dx == 0:
                    if first:
                        eng.tensor_mul(a[:, :], v[:, :], kts[t][:, :])
                    else:
                        eng.tensor_mul(tmp[:, :], v[:, :], kts[t][:, :])
                        eng.tensor_add(a[:, :], a[:, :], tmp[:, :])
                else:
                    a3, v3, k3 = _r3(a[:, :]), _r3(v[:, :]), _r3(kts[t][:, :])
                    tsl = _r3(tmp[:, :])[:, :, 0 : W - 1]
                    if dx < 0:
                        asl, vsl, ksl = a3[:, :, 1:W], v3[:, :, 0 : W - 1], k3[:, :, 1:W]
                    else:
                        asl, vsl, ksl = a3[:, :, 0 : W - 1], v3[:, :, 1:W], k3[:, :, 0 : W - 1]
                    eng.tensor_mul(tsl, vsl, ksl)
                    eng.tensor_add(asl, asl, tsl)
                first = False
            return a

        out_ap = out[c].rearrange("(b p) x -> p b x", p=P)
        if c < C - 1:
            acc_a = run_chain(CHAIN_A, "acc")
            nc.gpsimd.dma_start(out_ap, _r3(acc_a[:, :])).then_inc(store_sem, 16)
            acc_b = run_chain(CHAIN_B, "accb")
            nc.gpsimd.dma_start(
                out_ap, _r3(acc_b[:, :]), accum_op=mybir.AluOpType.add
            )._wait_ge(store_sem, 16 * (c + 1))
            continue
        acc = run_chain(TAP_ORDER, "acc")

        # Store via SWDGE (gpsimd) — a third DMA queue, so the store's
        # wait-for-compute never blocks the HWDGE load rings.
        nc.gpsimd.dma_start(out_ap, _r3(acc[:, :]))


_NC_CACHE = []


def _build():
    nc = bacc.Bacc(
        "TRN2",
        target_bir_lowering=False,
        debug=False,
        enable_asserts=True,
        num_devices=1,
    )
    with tile.TileContext(nc) as tc:
        with ExitStack() as ctx:
            _emit(nc, tc, ctx)
    nc.compile()
    return nc


def kernel(img, kernels):
    """img: [8, 3, 512, 512] f32; kernels: [8, 3, 9, 512, 512] f32.
    Returns [8, 3, 512, 512] f32."""
    first_call = not _NC_CACHE
    if first_call:
        _NC_CACHE.append(_build())
    nc = _NC_CACHE[0]
    img = np.asarray(img, dtype=np.float32)
    kernels = np.asarray(kernels, dtype=np.float32)
    in_maps = [
        {
            "img": np.ascontiguousarray(img[b]),
            "kernels": np.ascontiguousarray(kernels[b]),
        }
        for b in range(NCORES)
    ]
    if first_call:
        # Warm-up execution: the very first run after a fresh NEFF
        # compile/load was observed to occasionally return stale output.
        run_bass_kernel_spmd(nc, in_maps, core_ids=list(range(NCORES)))
    res = run_bass_kernel_spmd(nc, in_maps, core_ids=list(range(NCORES)))
    return np.stack([res.results[b]["out"] for b in range(NCORES)], axis=0)



# revision 3
# speedup vs baseline: 1.0007x; 1.0007x over previous
"""Dynamic per-pixel 3x3 filtering on 8 Trainium2 NeuronCores.

out[b,c,y,x] = sum_{ki,kj} img[b,c,y+ki-1,x+kj-1] * kernels[b,c,ki*3+kj,y,x]
(zero padding outside the image).

Sharding: pure data parallel, one batch sample per core (B=8, 8 cores).

Per-core layout: partition p holds 4 CONSECUTIVE image rows 4p..4p+3
(8 KB contiguous per partition -> single-descriptor DMAs). With this
layout a +-1 row shift is a FREE-DIM shift inside an extended tile
ext[p, bb, xx] = img[4p+bb-1, xx-1] (6 rows x 514 cols, zero padded),
so no TensorE transpose machinery / PSUM evacuation is needed; the two
boundary rows per partition are re-read from HBM (+0.5 MB/channel).

All elementwise work runs on DVE in fp16: TensorTensor supports the
2x_1p perf mode only when every operand is a packed 2-byte dtype, which
doubles throughput vs f32 (measured 2287 ns -> ~1190 ns per [128,2048]
pass). fp16 chain accumulation keeps max rel err ~1.6e-3, well under
the 2e-2 gate (bf16 would be ~1.3e-2 - too close). The ACT engine,
freed from PSUM evacuation, does the f32->fp16 casts of the streamed
kernel taps; it is the only engine with spare cycles for them.

Engine/queue split: kernel-tap loads on the SP (sync) HWDGE ring, img
loads + output stores on the gpsimd SWDGE ring, casts on ACT, taps on
DVE, TensorE idle. Output is stored as fp16 (halves store traffic) and
widened to f32 on the host.
"""

from contextlib import ExitStack

import numpy as np

import concourse.bacc as bacc
import concourse.mybir as mybir
import concourse.tile as tile
from concourse.bass_utils import run_bass_kernel_spmd

C, H, W = 3, 512, 512
KK = 9
NCORES = 8
P = 128
RPB = H // P         # 4 rows per partition
FW = RPB * W         # 2048 free-dim elems of a channel tile
EXT_W = W + 2        # 514: row length incl. zero pad cols
NG = 3               # kernel taps per load/cast chunk
F32 = mybir.dt.float32
F16 = mybir.dt.float16


def _r3(ap):
    """[128, FW] -> [128, RPB, W] row-block view of a channel tile."""
    return ap.rearrange("p (b x) -> p b x", x=W)


def _emit(nc, tc, ctx):
    img = nc.dram_tensor("img", (C, H, W), F32, kind="ExternalInput").ap()
    ker = nc.dram_tensor("kernels", (C, KK, H, W), F32, kind="ExternalInput").ap()
    out = nc.dram_tensor("out", (C, H, W), F16, kind="ExternalOutput").ap()

    s_pool = ctx.enter_context(tc.tile_pool(name="imgstage", bufs=2))
    e_pool = ctx.enter_context(tc.tile_pool(name="ext", bufs=2))
    kst_pool = ctx.enter_context(tc.tile_pool(name="kstage", bufs=2))
    kt_pool = ctx.enter_context(tc.tile_pool(name="kt", bufs=6))
    acc_pool = ctx.enter_context(tc.tile_pool(name="acc", bufs=2))
    tmp_pool = ctx.enter_context(tc.tile_pool(name="tmp", bufs=2))

    for c in range(C):
        # --- image: S[p, j, x] = img[c, 4p-1+j, x]  (f32 staging) ---
        S = s_pool.tile([P, 6, W], F32, tag="S")
        # Compute engines must start at an aligned partition, so zero the
        # full boundary slots first; the DMAs below overwrite the valid
        # 127 partitions, leaving the out-of-image row zero.
        nc.gpsimd.memset(S[:, 0, :], 0.0)
        nc.gpsimd.memset(S[:, 5, :], 0.0)
        rows4 = img[c].rearrange("(p b) x -> p b x", b=RPB)
        nc.gpsimd.dma_start(S[:, 1:5, :], rows4)
        # row 4p-1 for p>=1: img rows 3,7,...,507
        topsrc = img[c][3:511].rearrange("(p b) x -> p b x", b=RPB)
        nc.gpsimd.dma_start(S[1:128, 0, :], topsrc[:, 0, :])
        # row 4p+4 for p<=126: img rows 4,8,...,508
        botsrc = img[c][4:512].rearrange("(p b) x -> p b x", b=RPB)
        nc.gpsimd.dma_start(S[0:127, 5, :], botsrc[:, 0, :])

        # --- ext: fp16, zero pad cols at xx=0 and xx=513 ---
        ext = e_pool.tile([P, 6, EXT_W], F16, tag="ext")
        nc.gpsimd.memset(ext[:, :, 0:1], 0.0)
        nc.gpsimd.memset(ext[:, :, EXT_W - 1 : EXT_W], 0.0)
        nc.scalar.copy(ext[:, :, 1 : W + 1], S[:, :, :])

        # --- kernel taps: stream in NG-tap chunks, cast f32 -> fp16 ---
        kall = ker[c].rearrange("t (p b) x -> p t (b x)", b=RPB)
        kts = []
        for g in range(KK // NG):
            kst = kst_pool.tile([P, NG, FW], F32, tag="kst")
            nc.sync.dma_start(kst[:, :, :], kall[:, g * NG : (g + 1) * NG, :])
            kt = kt_pool.tile([P, NG, FW], F16, tag="kt")
            nc.scalar.copy(kt[:, :, :], kst[:, :, :])
            kts.append(kt)

        # --- 9 taps on DVE, all fp16 (2x_1p) ---
        acc = acc_pool.tile([P, FW], F16, tag="acc")
        tmp = tmp_pool.tile([P, FW], F16, tag="tmp")
        for t in range(KK):
            ki, kj = divmod(t, 3)
            v = ext[:, ki : ki + RPB, kj : kj + W]
            ktap = _r3(kts[t // NG][:, t % NG, :])
            if t == 0:
                nc.vector.tensor_mul(_r3(acc[:, :]), v, ktap)
            else:
                nc.vector.tensor_mul(_r3(tmp[:, :]), v, ktap)
                nc.vector.tensor_add(acc[:, :], acc[:, :], tmp[:, :])

        # --- store fp16 ---
        nc.gpsimd.dma_start(
            out[c].rearrange("(p b) x -> p (b x)", b=RPB), acc[:, :]
        )


_NC_CACHE = []


def _build():
    nc = bacc.Bacc(
        "TRN2",
        target_bir_lowering=False,
        debug=False,
        enable_asserts=True,
        num_devices=1,
    )
    with tile.TileContext(nc) as tc:
        with ExitStack() as ctx:
            _emit(nc, tc, ctx)
    nc.compile()
    return nc


def kernel(img, kernels):
    """img: [8, 3, 512, 512] f32; kernels: [8, 3, 9, 512, 512] f32.
    Returns [8, 3, 512, 512] f32."""
    first_call = not _NC_CACHE
    if first_call:
        _NC_CACHE.append(_build())
    nc = _NC_CACHE[0]
    img = np.asarray(img, dtype=np.float32)
    kernels = np.asarray(kernels, dtype=np.float32)
    in_maps = [
        {
            "img": np.ascontiguousarray(img[b]),
            "kernels": np.ascontiguousarray(kernels[b]),
        }
        for b in range(NCORES)
    ]
    if first_call:
        # Warm-up execution: the very first run after a fresh NEFF
        # compile/load was observed to occasionally return stale output.
        run_bass_kernel_spmd(nc, in_maps, core_ids=list(range(NCORES)))
    res = run_bass_kernel_spmd(nc, in_maps, core_ids=list(range(NCORES)))
    return np.stack(
        [np.asarray(res.results[b]["out"], dtype=np.float32) for b in range(NCORES)],
        axis=0,
    )


# revision 6
# speedup vs baseline: 1.1897x; 1.1888x over previous
"""Dynamic per-pixel 3x3 filtering on 8 Trainium2 NeuronCores.

out[b,c,y,x] = sum_{ki,kj} img[b,c,y+ki-1,x+kj-1] * kernels[b,c,ki*3+kj,y,x]
(zero padding outside the image).

Sharding: pure data parallel, one batch sample per core (B=8, 8 cores).

Per-core layout: partition p holds 4 CONSECUTIVE image rows 4p..4p+3
(8 KB contiguous per partition -> single-descriptor DMAs). With this
layout a +-1 row shift is a FREE-DIM shift inside an extended tile
ext[p, bb, xx] = img[4p+bb-1, xx-1] (6 rows x 514 cols, zero padded),
so no TensorE transpose machinery / PSUM evacuation is needed; the
overlapping 6-row window is loaded as ONE 12KB-per-partition DMA
(re-reads 2 of 6 rows: +0.5 MB/channel on a 34 MB budget).

All elementwise work runs on DVE in fp16: TensorTensor supports the
2x_1p perf mode only when every operand is a packed 2-byte dtype, which
doubles throughput vs f32 (measured 2287 -> 1224 ns per [128,2048]
pass). fp16 chain accumulation keeps max rel err ~1.2e-3, well under
the 2e-2 gate (bf16 would be ~1.3e-2 - too close). The ACT engine,
freed from PSUM evacuation, does the f32->fp16 casts of the streamed
kernel taps.

The DMA fleet is HBM-bound (~22 GB/s per SDMA engine, 16 engines), so
the schedule aims to keep all engines fed: kernel taps stream as nine
1 MB single-descriptor-per-partition DMAs per channel on the SP HWDGE
ring, img windows on the ACT HWDGE ring, stores on the gpsimd SWDGE
ring. Output is stored fp16 (halves store traffic), widened on host.
"""

from contextlib import ExitStack

import numpy as np

import concourse.bacc as bacc
import concourse.mybir as mybir
import concourse.tile as tile
from concourse.ap import AP
from concourse.bass_utils import run_bass_kernel_spmd

C, H, W = 3, 512, 512
KK = 9
NCORES = 8
P = 128
RPB = H // P         # 4 rows per partition
FW = RPB * W         # 2048 free-dim elems of a channel tile
EXT_W = W + 2        # 514: row length incl. zero pad cols
F32 = mybir.dt.float32
F16 = mybir.dt.float16


def _r3(ap):
    """[128, FW] -> [128, RPB, W] row-block view of a channel tile."""
    return ap.rearrange("p (b x) -> p b x", x=W)


def _emit(nc, tc, ctx):
    img = nc.dram_tensor("img", (C, H, W), F32, kind="ExternalInput").ap()
    ker = nc.dram_tensor("kernels", (C, KK, H, W), F32, kind="ExternalInput").ap()
    out = nc.dram_tensor("out", (C, H, W), F16, kind="ExternalOutput").ap()

    s_pool = ctx.enter_context(tc.tile_pool(name="imgstage", bufs=2))
    e_pool = ctx.enter_context(tc.tile_pool(name="ext", bufs=2))
    kst_pool = ctx.enter_context(tc.tile_pool(name="kstage", bufs=4))
    kt_pool = ctx.enter_context(tc.tile_pool(name="kt", bufs=12))
    acc_pool = ctx.enter_context(tc.tile_pool(name="acc", bufs=2))
    tmp_pool = ctx.enter_context(tc.tile_pool(name="tmp", bufs=3))

    for c in range(C):
        # --- image: S[p, j, x] = img[c, 4p-1+j, x]  (f32 staging) ---
        S = s_pool.tile([P, 6, W], F32, tag="S")
        # Out-of-image rows must read as zero. Compute-engine accesses must
        # start on an aligned partition, so zero the full row-slots; the
        # DMAs below overwrite every valid partition.
        nc.gpsimd.memset(S[:, 0, :], 0.0)
        nc.gpsimd.memset(S[:, 5, :], 0.0)
        # One overlapping-window DMA for p=1..126: rows 4p-1 .. 4p+4
        # (12 KB contiguous per partition; re-reads 2 of the 6 rows).
        base = img[c]
        win = AP(base.tensor, c * H * W + 3 * W, [[RPB * W, 126], [W, 6], [1, W]])
        nc.scalar.dma_start(S[1:127, :, :], win)
        # p=0: rows 0..4 into slots 1..5 (slot 0 stays zero)
        nc.scalar.dma_start(
            S[0:1, 1:6, :], AP(base.tensor, c * H * W, [[W, 1], [W, 5], [1, W]])
        )
        # p=127: rows 507..511 into slots 0..4 (slot 5 stays zero)
        nc.scalar.dma_start(
            S[127:128, 0:5, :],
            AP(base.tensor, c * H * W + 507 * W, [[W, 1], [W, 5], [1, W]]),
        )

        # --- ext: fp16, zero pad cols at xx=0 and xx=513 ---
        ext = e_pool.tile([P, 6, EXT_W], F16, tag="ext")
        nc.gpsimd.memset(ext[:, :, 0:1], 0.0)
        nc.gpsimd.memset(ext[:, :, EXT_W - 1 : EXT_W], 0.0)
        nc.scalar.copy(ext[:, :, 1 : W + 1], S[:, :, :])

        # --- kernel taps: stream one tap at a time, cast f32 -> fp16 ---
        kall = ker[c].rearrange("t (p b) x -> p t (b x)", b=RPB)
        kts = []
        for t in range(KK):
            kst = kst_pool.tile([P, FW], F32, tag="kst")
            nc.sync.dma_start(kst[:, :], kall[:, t, :])
            kt = kt_pool.tile([P, FW], F16, tag="kt")
            nc.scalar.copy(kt[:, :], kst[:, :])
            kts.append(kt)

        # --- 9 taps on DVE, all fp16 (2x_1p) ---
        acc = acc_pool.tile([P, FW], F16, tag="acc")
        for t in range(KK):
            ki, kj = divmod(t, 3)
            v = ext[:, ki : ki + RPB, kj : kj + W]
            ktap = _r3(kts[t])
            if t == 0:
                nc.vector.tensor_mul(_r3(acc[:, :]), v, ktap)
            else:
                tmp = tmp_pool.tile([P, FW], F16, tag="tmp")
                nc.vector.tensor_mul(_r3(tmp[:, :]), v, ktap)
                nc.vector.tensor_add(acc[:, :], acc[:, :], tmp[:, :])

        # --- store fp16 ---
        nc.gpsimd.dma_start(
            out[c].rearrange("(p b) x -> p (b x)", b=RPB), acc[:, :]
        )


_NC_CACHE = []


def _build():
    nc = bacc.Bacc(
        "TRN2",
        target_bir_lowering=False,
        debug=False,
        enable_asserts=True,
        num_devices=1,
    )
    with tile.TileContext(nc) as tc:
        with ExitStack() as ctx:
            _emit(nc, tc, ctx)
    nc.compile()
    return nc


def kernel(img, kernels):
    """img: [8, 3, 512, 512] f32; kernels: [8, 3, 9, 512, 512] f32.
    Returns [8, 3, 512, 512] f32."""
    first_call = not _NC_CACHE
    if first_call:
        _NC_CACHE.append(_build())
    nc = _NC_CACHE[0]
    img = np.asarray(img, dtype=np.float32)
    kernels = np.asarray(kernels, dtype=np.float32)
    in_maps = [
        {
            "img": np.ascontiguousarray(img[b]),
            "kernels": np.ascontiguousarray(kernels[b]),
        }
        for b in range(NCORES)
    ]
    if first_call:
        # Warm-up execution: the very first run after a fresh NEFF
        # compile/load was observed to occasionally return stale output.
        run_bass_kernel_spmd(nc, in_maps, core_ids=list(range(NCORES)))
    res = run_bass_kernel_spmd(nc, in_maps, core_ids=list(range(NCORES)))
    return np.stack(
        [np.asarray(res.results[b]["out"], dtype=np.float32) for b in range(NCORES)],
        axis=0,
    )


# revision 8
# speedup vs baseline: 1.3284x; 1.1166x over previous
"""Dynamic per-pixel 3x3 filtering on 8 Trainium2 NeuronCores.

out[b,c,y,x] = sum_{ki,kj} img[b,c,y+ki-1,x+kj-1] * kernels[b,c,ki*3+kj,y,x]
(zero padding outside the image).

Sharding: pure data parallel, one batch sample per core (B=8, 8 cores).

Per-core layout: partition p holds 4 CONSECUTIVE image rows 4p..4p+3
(8 KB contiguous per partition -> single-descriptor DMAs). A +-1 row
shift is then a FREE-DIM shift inside an extended tile
ext[p, bb, xx] = img[4p+bb-1, xx-1] (6 rows x 514 cols, zero padded).
The two boundary rows (4p-1, 4p+4) live on the neighbouring partition,
so they are produced by the otherwise-idle TensorE as a +-1 partition
shift: a matmul with a shifted identity as the stationary operand
(edge partitions zero-fill automatically), evacuated PSUM->SBUF by ACT
with a free f32->fp16 cast. This keeps img HBM traffic at exactly
1 MB/channel instead of re-reading shifted rows.

All elementwise work runs on DVE in fp16: TensorTensor supports the
2x_1p perf mode only when every operand is a packed 2-byte dtype,
doubling throughput vs f32 (measured 2287 -> 1221 ns per [128,2048]
pass). fp16 chain accumulation keeps max rel err ~1.2e-3, well under
the 2e-2 gate. ACT does the f32->fp16 casts of the streamed kernel
taps.

The DMA fleet is HBM-bound (~22 GB/s per SDMA engine x16), so traffic
is minimal and spread: kernel taps stream as nine 1 MB single-
descriptor-per-partition DMAs per channel on the SP HWDGE ring, img on
the ACT HWDGE ring, stores on the gpsimd SWDGE ring. Output is stored
fp16 (halves store traffic), widened on host. The last channel's final
tap and store are split in half so the post-last-DMA pipeline drain
(cast -> mult -> add -> store) runs at half tile size.
"""

from contextlib import ExitStack

import numpy as np

import concourse.bacc as bacc
import concourse.mybir as mybir
import concourse.tile as tile
from concourse import masks
from concourse.bass_utils import run_bass_kernel_spmd

C, H, W = 3, 512, 512
KK = 9
NCORES = 8
P = 128
RPB = H // P         # 4 rows per partition
FW = RPB * W         # 2048 free-dim elems of a channel tile
EXT_W = W + 2        # 514: row length incl. zero pad cols
F32 = mybir.dt.float32
F16 = mybir.dt.float16


def _r3(ap):
    """[128, n*W] -> [128, n, W] row-block view."""
    return ap.rearrange("p (b x) -> p b x", x=W)


def _emit(nc, tc, ctx):
    img = nc.dram_tensor("img", (C, H, W), F32, kind="ExternalInput").ap()
    ker = nc.dram_tensor("kernels", (C, KK, H, W), F32, kind="ExternalInput").ap()
    out = nc.dram_tensor("out", (C, H, W), F16, kind="ExternalOutput").ap()

    s_pool = ctx.enter_context(tc.tile_pool(name="imgstage", bufs=2))
    e_pool = ctx.enter_context(tc.tile_pool(name="ext", bufs=2))
    kst_pool = ctx.enter_context(tc.tile_pool(name="kstage", bufs=4))
    kt_pool = ctx.enter_context(tc.tile_pool(name="kt", bufs=12))
    acc_pool = ctx.enter_context(tc.tile_pool(name="acc", bufs=2))
    tmp_pool = ctx.enter_context(tc.tile_pool(name="tmp", bufs=3))
    ps_pool = ctx.enter_context(tc.tile_pool(name="ps", bufs=4, space="PSUM"))
    id_pool = ctx.enter_context(tc.tile_pool(name="ident", bufs=1))

    # Shifted identities for TensorE partition shifts (as lhsT):
    # up[q, m] = 1 iff m == q+1  -> out[m] = rhs[m-1]   (row 4m-1 from 4p+3)
    # dn[q, m] = 1 iff m == q-1  -> out[m] = rhs[m+1]   (row 4m+4 from 4p)
    idc = id_pool.tile([P, 2, P + 1], F32, tag="idc")
    nc.gpsimd.memset(idc[:, :, :], 0.0)
    masks.make_identity(nc, idc[:, 0, 1 : P + 1], nomemset=True)
    masks.make_identity(nc, idc[:, 1, 0:P], nomemset=True)
    up = idc[:, 0, 0:P]
    dn = idc[:, 1, 1 : P + 1]

    for c in range(C):
        # --- image mid rows: S[p, b, x] = img[c, 4p+b, x]  (f32) ---
        S = s_pool.tile([P, RPB, W], F32, tag="S")
        nc.scalar.dma_start(S[:, :, :], img[c].rearrange("(p b) x -> p b x", b=RPB))

        # --- ext: fp16 [128, 6, 514], zero pad cols ---
        ext = e_pool.tile([P, 6, EXT_W], F16, tag="ext")
        nc.gpsimd.memset(ext[:, :, 0:1], 0.0)
        nc.gpsimd.memset(ext[:, :, EXT_W - 1 : EXT_W], 0.0)
        nc.scalar.copy(ext[:, 1:5, 1 : W + 1], S[:, :, :])
        # boundary rows via TensorE partition shift, evac + cast on ACT
        ps_t = ps_pool.tile([P, W], F32, tag="ps")
        nc.tensor.matmul(ps_t[:, :], up, S[:, 3, :], start=True, stop=True)
        nc.scalar.copy(ext[:, 0, 1 : W + 1], ps_t[:, :])
        ps_b = ps_pool.tile([P, W], F32, tag="ps")
        nc.tensor.matmul(ps_b[:, :], dn, S[:, 0, :], start=True, stop=True)
        nc.scalar.copy(ext[:, 5, 1 : W + 1], ps_b[:, :])

        # --- kernel taps: stream one tap at a time, cast f32 -> fp16 ---
        kall = ker[c].rearrange("t (p b) x -> p t (b x)", b=RPB)
        last = c == C - 1

        acc = acc_pool.tile([P, FW], F16, tag="acc")
        for t in range(KK):
            ki, kj = divmod(t, 3)
            # Split the very last tap (and its load/cast) in half so the
            # end-of-kernel drain works on half-size tiles.
            halves = 2 if (last and t == KK - 1) else 1
            for h in range(halves):
                n_b = RPB // halves
                b0 = h * n_b
                sl = slice(b0 * W, (b0 + n_b) * W)
                kst = kst_pool.tile([P, FW], F32, tag="kst")
                nc.sync.dma_start(kst[:, sl], kall[:, t, sl])
                kt = kt_pool.tile([P, FW], F16, tag="kt")
                nc.scalar.copy(kt[:, sl], kst[:, sl])
                v = ext[:, ki + b0 : ki + b0 + n_b, kj : kj + W]
                ktap = _r3(kt[:, sl])
                if t == 0:
                    nc.vector.tensor_mul(_r3(acc[:, sl]), v, ktap)
                else:
                    tmp = tmp_pool.tile([P, FW], F16, tag="tmp")
                    nc.vector.tensor_mul(_r3(tmp[:, sl]), v, ktap)
                    nc.vector.tensor_add(acc[:, sl], acc[:, sl], tmp[:, sl])
                if last and t == KK - 1:
                    nc.gpsimd.dma_start(
                        out[c].rearrange("(p b) x -> p (b x)", b=RPB)[:, sl],
                        acc[:, sl],
                    )
        if not last:
            nc.gpsimd.dma_start(
                out[c].rearrange("(p b) x -> p (b x)", b=RPB), acc[:, :]
            )


_NC_CACHE = []


def _build():
    nc = bacc.Bacc(
        "TRN2",
        target_bir_lowering=False,
        debug=False,
        enable_asserts=True,
        num_devices=1,
    )
    with tile.TileContext(nc) as tc:
        with ExitStack() as ctx:
            _emit(nc, tc, ctx)
    nc.compile()
    return nc


def kernel(img, kernels):
    """img: [8, 3, 512, 512] f32; kernels: [8, 3, 9, 512, 512] f32.
    Returns [8, 3, 512, 512] f32."""
    first_call = not _NC_CACHE
    if first_call:
        _NC_CACHE.append(_build())
    nc = _NC_CACHE[0]
    img = np.asarray(img, dtype=np.float32)
    kernels = np.asarray(kernels, dtype=np.float32)
    in_maps = [
        {
            "img": np.ascontiguousarray(img[b]),
            "kernels": np.ascontiguousarray(kernels[b]),
        }
        for b in range(NCORES)
    ]
    if first_call:
        # Warm-up execution: the very first run after a fresh NEFF
        # compile/load was observed to occasionally return stale output.
        run_bass_kernel_spmd(nc, in_maps, core_ids=list(range(NCORES)))
    res = run_bass_kernel_spmd(nc, in_maps, core_ids=list(range(NCORES)))
    return np.stack(
        [np.asarray(res.results[b]["out"], dtype=np.float32) for b in range(NCORES)],
        axis=0,
    )
